# revision 32
# baseline (speedup 1.0000x reference)
"""Multi-head causal attention with RoPE on 8 Trainium2 NeuronCores.

Problem: x[2, 2048, 1024], 16 heads, d_k=64, RoPE(theta=1e4), causal,
weights W{q,k,v,o}[1024, 1024] stored [d_out, d_in].

Sharding: 2 batches x 4 head-groups -> 8 cores. Core c handles batch
c//4, heads 4*(c%4) .. 4*(c%4)+4. Each core computes its 4 heads'
attention plus the partial o_proj for its head columns; the host sums
the 4 partials per batch (the "all-reduce after o_proj").

Two-phase structure (v3), one PSUM pool, banks rotating via tags:
  Phase 1 (all 4 chunks): Q/K projection psum pairs on SA/SB
    ([128,2,SC], 2 banks each), V on AV0/AV1. Chunk c+1's projection
    matmuls wait only on chunk c's RoPE psum muls (SA/SB WAR), so the
    PE pipelines across chunks. RoPE cross-parity swaps are SBUF
    copies split 3:1 ACT:DVE per tile (GpSimd measured ~1.9us/copy —
    unusable).
  Phase 2 (kt loops qc=0..3): score psum per head-pair on SA/SB with
    ONE exp activation per pair per k-tile ([128, 2, W] across both
    banks), A/B staggered so scores(kt+1) overlap exp(kt); attn@V
    accumulates into AV0..AV3, lagging one kt. The denominator
    broadcast (rbp) and the previous chunk's o_proj psum ride AV2/AV3
    after normalize frees them, so the SA/SB score ring is never
    blocked; o_proj(qc-1) is emitted after qc's first two kt
    emissions so ACT keeps streaming exps across the qc boundary.
    Output stores stream per 128-row s-tile as bf16.
"""

import sys

if "/opt/trn_rl_repo" not in sys.path:
    sys.path.insert(0, "/opt/trn_rl_repo")

import numpy as np

import concourse.bass as bass
import concourse.mybir as mybir
import concourse.tile as tile
from concourse import bacc
from concourse.bass_utils import run_bass_kernel_spmd

F32 = mybir.dt.float32
BF16 = mybir.dt.bfloat16
EXP = mybir.ActivationFunctionType.Exp

B = 2
S = 2048
D = 1024
H = 16
DK = 64
HC = 4          # heads per core
E = HC * DK     # 256 d_out columns per core
THETA = 10000.0
SC = 512        # seq chunk (psum free dim)
NSC = S // SC   # 4
NST = S // 128  # 16 s-tiles
NEG = -1.0e30

_COMPILED = None


def _build():
    nc = bacc.Bacc("TRN2", target_bir_lowering=False, debug=False, num_devices=8)

    # weights/x are pre-packed host-side into the exact SBUF layouts so
    # every DMA segment is multi-KB contiguous (the (c p) rearrange
    # loads had 512B segments and cost a ~20us lead-in)
    xT = nc.dram_tensor("xT", [128, NSC, 8, SC], BF16, kind="ExternalInput")
    wqT = nc.dram_tensor("wqT", [128, 8, E], BF16, kind="ExternalInput")
    wkT = nc.dram_tensor("wkT", [128, 8, E], BF16, kind="ExternalInput")
    wvT = nc.dram_tensor("wvT", [128, 8, E], BF16, kind="ExternalInput")
    woT = nc.dram_tensor("woT", [128, 2, D], BF16, kind="ExternalInput")
    cosT = nc.dram_tensor("cosT", [128, S], F32, kind="ExternalInput")
    sinT = nc.dram_tensor("sinT", [128, S], F32, kind="ExternalInput")
    eye = nc.dram_tensor("eye", [128, 128], BF16, kind="ExternalInput")
    tri = nc.dram_tensor("tri", [128, 128], BF16, kind="ExternalInput")
    sel = nc.dram_tensor("sel", [128, 256], BF16, kind="ExternalInput")
    out_d = nc.dram_tensor("out", [S, D], BF16, kind="ExternalOutput")

    with tile.TileContext(nc) as tc:
        with (
            tc.tile_pool(name="const", bufs=1) as const,
            tc.tile_pool(name="persist", bufs=1) as persist,
            tc.tile_pool(name="xp", bufs=2) as xp,
            tc.tile_pool(name="ropet", bufs=2) as ropet,
            tc.tile_pool(name="cspool", bufs=2) as cspool,
            tc.tile_pool(name="expool", bufs=3) as expool,
            tc.tile_pool(name="sopool", bufs=3) as sopool,
            tc.tile_pool(name="ps", bufs=1, space="PSUM") as ps,
        ):
            # ---- constant loads ------------------------------------
            wq_sb = const.tile([128, 8, E], BF16)
            wk_sb = const.tile([128, 8, E], BF16)
            wv_sb = const.tile([128, 8, E], BF16)
            wo_sb = const.tile([128, 2, D], BF16)
            eye_sb = const.tile([128, 128], BF16)
            tri_sb = const.tile([128, 128], BF16)
            sel_sb = const.tile([128, 2, 128], BF16)
            nc.sync.dma_start(wq_sb[:], wqT[:])

            # ---- persistent activations ----------------------------
            qA = persist.tile([128, S], BF16)   # heads 0,1 (rows 64h+32p+j)
            qB = persist.tile([128, S], BF16)   # heads 2,3
            kA = persist.tile([128, S], BF16)
            kB = persist.tile([128, S], BF16)
            v_sb = persist.tile([128, NST, HC * 65], BF16)
            ao = persist.tile([128, 2, S], BF16)    # o_proj lhsT
            # den slot for head h: (partition, block) = (32h, 0) for
            # h<3, (0, 1) for h=3 — matmul bases must be 0/32/64.
            den = persist.tile([128, SC], F32)
            rden = persist.tile([128, SC], F32)
            rdenb = persist.tile([128, SC], BF16)
            dume = persist.tile([128, 1], F32)

            v3 = v_sb[:].rearrange("p t (h c) -> p t h c", c=65)
            # ones column built on-device (a DMA of this strided
            # region degenerates to 8192 two-byte descriptors that
            # congest the queues for ~10us at startup)
            nc.vector.memset(v3[:, :, :, 64:65], 1.0)
            nc.vector.memset(den[:], 1.0)  # unused rows stay recip-safe
            # prefetch the exp table-set load during the proj phase
            nc.scalar.activation(dume[:], den[:, 0:1], EXP)

            qk = ((qA, kA), (qB, kB))

            def proj_chunk(c):
                """QKV projections + RoPE + V layout for chunk c."""
                sl = slice(SC * c, SC * (c + 1))
                # chunk-0 triggers spread across engine queues so they
                # issue in parallel (Sync serializes at ~0.7-7us each)
                x_sb = xp.tile([128, 8, SC], BF16, name=f"x_{c}", tag="x")
                (nc.scalar if c == 0 else nc.sync).dma_start(
                    x_sb[:], xT[:, c])
                cs_sb = cspool.tile([128, SC], F32, name=f"cos_{c}",
                                    tag="cos")
                sn_sb = cspool.tile([128, SC], F32, name=f"sin_{c}",
                                    tag="sin")
                (nc.gpsimd if c == 0 else nc.sync).dma_start(
                    cs_sb[:], cosT[:, sl])
                (nc.gpsimd if c == 0 else nc.sync).dma_start(
                    sn_sb[:], sinT[:, sl])
                if c == 0:
                    nc.gpsimd.dma_start(wk_sb[:], wkT[:])
                    nc.gpsimd.dma_start(wv_sb[:], wvT[:])
                elif c == 1:
                    nc.sync.dma_start(eye_sb[:], eye[:])
                    nc.sync.dma_start(tri_sb[:], tri[:])
                    nc.sync.dma_start(
                        sel_sb[:],
                        sel[:].rearrange("p (c q) -> p c q", c=2))
                    nc.sync.dma_start(wo_sb[:], woT[:])

                pq = [ps.tile([128, SC], F32, name=f"pq{t}_{c}",
                              tag=f"SC{t}") for t in range(2)]
                pk = [ps.tile([128, SC], F32, name=f"pk{t}_{c}",
                              tag=f"SC{2 + t}") for t in range(2)]
                for psrc, w_sb in ((pq, wq_sb), (pk, wk_sb)):
                    for t in range(2):
                        es = slice(128 * t, 128 * (t + 1))
                        for dc in range(8):
                            nc.tensor.matmul(
                                psrc[t][:], w_sb[:, dc, es],
                                x_sb[:, dc, :],
                                start=(dc == 0), stop=(dc == 7))

                # RoPE: row = 64*(h%2) + 32*parity + j within each tile.
                # sinT carries +sin on parity-0 rows and -sin on
                # parity-1 rows, so after swapping 32-row blocks of
                # t1 = psum*sinAlt within each 64-row head block:
                #   t1s[p0] = -x2*s, t1s[p1] = +x1*s
                # and the combine is ONE aligned add: out = t0+t1s.
                # Cross-partition swaps are SBUF copies, 3 ACT + 1 DVE
                # per tile (GpSimd measured ~1.9us/copy — unusable).
                C = cs_sb[:]
                Sn = sn_sb[:]
                for name, psrc, dsts in (
                    ("q", pq, (qA, qB)),
                    ("k", pk, (kA, kB)),
                ):
                    for t in range(2):
                        dst = dsts[t]
                        t0 = ropet.tile([128, SC], F32,
                                        name=f"t0{name}{t}_{c}",
                                        tag=f"t0{name}{t}")
                        t1 = ropet.tile([128, SC], F32,
                                        name=f"t1{name}{t}_{c}",
                                        tag=f"t1{name}{t}")
                        t1s = ropet.tile([128, SC], F32,
                                         name=f"t1s{name}{t}_{c}",
                                         tag=f"t1s{name}{t}")
                        nc.vector.tensor_mul(t0[:], psrc[t][:], C)
                        nc.vector.tensor_mul(t1[:], psrc[t][:], Sn)
                        for bb in range(4):
                            dsl = slice(32 * bb, 32 * bb + 32)
                            ssl2 = slice(32 * (bb ^ 1), 32 * (bb ^ 1) + 32)
                            if bb == 3:
                                nc.vector.tensor_copy(
                                    t1s[dsl, :], t1[ssl2, :])
                            else:
                                nc.scalar.copy(t1s[dsl, :], t1[ssl2, :])
                        nc.vector.tensor_add(dst[:, sl], t0[:], t1s[:])

                # V projection into [k, h*65+dk] layout (ones preset)
                pv = [ps.tile([128, 2, 256], F32, name=f"pv{t}_{c}",
                              tag=f"AV{t}") for t in range(2)]
                for st in range(4):
                    ssl = slice(128 * st, 128 * (st + 1))
                    for dc in range(8):
                        nc.tensor.matmul(
                            pv[st // 2][:, st % 2, :],
                            x_sb[:, dc, ssl], wv_sb[:, dc, :],
                            start=(dc == 0), stop=(dc == 7))
                for st in range(4):
                    nc.scalar.copy(
                        v3[:, 4 * c + st, :, 0:64],
                        pv[st // 2][:, st % 2, :]
                        .rearrange("p (h c) -> p h c", c=64))

            def o_proj_chunk(c, tags=("AV2", "AV3")):
                """o_proj partial for chunk c's 4 s-tiles (ao ready).

                po psum rides the given freed tags (AV2/AV3 inside the
                kt loops; all six free tags for the final chunk) so the
                score ring never waits on it and the po->evac chain is
                deep enough to pipeline."""
                for i in range(4):
                    stg = 4 * c + i
                    ssl = slice(128 * stg, 128 * (stg + 1))
                    for dc in range(2):
                        j = 2 * i + dc
                        po = ps.tile([128, SC], F32,
                                     name=f"po_{stg}_{dc}",
                                     tag=tags[j % len(tags)])
                        for pr in range(2):
                            nc.tensor.matmul(
                                po[:], ao[:, pr, ssl],
                                wo_sb[:, pr, 512 * dc:512 * (dc + 1)],
                                start=(pr == 0), stop=(pr == 1))
                        so = sopool.tile([128, SC], BF16,
                                         name=f"so_{stg}_{dc}", tag="so")
                        if j % 2 == 0:
                            nc.vector.tensor_copy(so[:], po[:])
                        else:
                            nc.scalar.copy(so[:], po[:])
                        nc.sync.dma_start(
                            out_d[ssl, 512 * dc:512 * (dc + 1)], so[:])

            def attention_chunk(qc):
                """Scores + exp + attn@V + normalize for q-chunk qc.

                o_proj for qc-1 is emitted after this chunk's first
                two kt emissions, so ACT keeps streaming exps across
                the qc boundary while the PE runs the po matmuls."""
                qsl = slice(SC * qc, SC * (qc + 1))
                nkt = 4 * qc + 4
                # avs allocated lazily AFTER o_proj(qc-1)'s po tiles so
                # the AV2/3 tag ring is: avs(qc-1), rbp(qc-1),
                # po(qc-1), avs(qc) — po never waits on this kt loop.
                avs = []

                def get_avs():
                    if not avs:
                        avs.extend(
                            ps.tile([128, SC], F32, name=f"av{h}_{qc}",
                                    tag=f"AV{h}") for h in range(HC))
                    return avs

                # diag k-tiles first: ends the chunk on uniform
                # full-width exps (measured ~3us better than natural
                # order — the shrinking diag tail skews the per-head
                # pipelines at the chunk boundary)
                kt_order = list(range(4 * qc, 4 * qc + 4)) + list(range(4 * qc))
                first_kt, last_kt = kt_order[0], kt_order[-1]
                prev = None

                def attn_v(pkt, pw, pexs, final=False):
                    for h in range(HC):
                        nc.tensor.matmul(
                            get_avs()[h][0:65, pw:SC],
                            v_sb[:, pkt, 65 * h:65 * h + 65],
                            pexs[h][:, pw:SC],
                            start=(pkt == first_kt), stop=(pkt == last_kt))
                        if final:
                            # per-head normalize pieces right behind
                            # the head's last attn@V, ACT/DVE split, so
                            # the tail chain parallelizes instead of
                            # serializing ~9us on DVE
                            u, pr = h % 2, h // 2
                            if h % 2 == 0:
                                nc.scalar.copy(
                                    den[32 * h:32 * h + 1, :],
                                    get_avs()[h][64:65, :])
                                nc.vector.tensor_copy(
                                    ao[64 * u:64 * u + 64, pr, qsl],
                                    get_avs()[h][0:64, :])
                            else:
                                nc.vector.tensor_copy(
                                    den[32 * h:32 * h + 1, :],
                                    get_avs()[h][64:65, :])
                                nc.scalar.copy(
                                    ao[64 * u:64 * u + 64, pr, qsl],
                                    get_avs()[h][0:64, :])

                for ki, kt in enumerate(kt_order):
                    ksl = slice(128 * kt, 128 * (kt + 1))
                    diag = kt >= 4 * qc
                    w = 128 * (kt - 4 * qc) if diag else 0
                    exs = []
                    # one 1-bank score tile + one exp per head: four
                    # independent exp->score chains keep ACT saturated
                    # no matter how the scheduler orders them (pair
                    # batching had a 2-bank WAR ping-pong the scheduler
                    # kept serializing)
                    for h in range(HC):
                        q_t, k_t = qk[h // 2]
                        rsl = slice(64 * (h % 2), 64 * (h % 2) + 64)
                        sc = ps.tile([128, SC], F32,
                                     name=f"sc{h}_{qc}_{kt}",
                                     tag=f"SC{h}")
                        nc.tensor.matmul(
                            sc[:, w:SC], k_t[rsl, ksl],
                            q_t[rsl, qsl][:, w:SC],
                            start=True, stop=not diag)
                        if diag:
                            nc.tensor.matmul(
                                sc[:, w:w + 128], eye_sb[:],
                                tri_sb[:], start=False, stop=True)
                        ex = expool.tile([128, SC], BF16,
                                         name=f"ex{h}_{qc}_{kt}",
                                         tag=f"ex{h}")
                        nc.scalar.activation(
                            ex[:, w:SC], sc[:, w:SC], EXP)
                        exs.append(ex)
                        if h == HC - 1 and prev is not None:
                            attn_v(*prev)
                    prev = (kt, w, exs)
                    if ki == 0 and qc > 0:
                        o_proj_chunk(qc - 1)
                attn_v(*prev, final=True)

                nc.vector.reciprocal_approx_fast(rden[:], den[:])
                nc.vector.tensor_copy(rdenb[:], rden[:])
                for pr in range(2):
                    rbp = ps.tile([128, SC], F32, name=f"rbp_{qc}_{pr}",
                                  tag=f"AV{2 + pr}")
                    nc.tensor.matmul(
                        rbp[:], sel_sb[:, pr, :], rdenb[:],
                        start=True, stop=True)
                    nc.vector.tensor_mul(
                        ao[:, pr, qsl], ao[:, pr, qsl], rbp[:])

            for c in range(NSC):
                proj_chunk(c)
            for qc in range(NSC):
                attention_chunk(qc)
            o_proj_chunk(
                NSC - 1,
                tags=("AV2", "AV3", "SC0", "SC1", "SC2", "SC3"))

    nc.compile()
    return nc


def _host_inputs(x, Wq, Wk, Wv, Wo, token_positions):
    """Build the 8 per-core input maps (all host-side numpy prep)."""
    import ml_dtypes

    x = np.asarray(x, dtype=np.float32)
    Wq = np.asarray(Wq, dtype=np.float32)
    Wk = np.asarray(Wk, dtype=np.float32)
    Wv = np.asarray(Wv, dtype=np.float32)
    Wo = np.asarray(Wo, dtype=np.float32)
    pos = np.asarray(token_positions, dtype=np.int64)

    # RoPE tables per batch: row 32a+j -> cos/sin(pos[s] * freq[j])
    j = np.arange(0, DK, 2, dtype=np.float64) / DK
    freq = 1.0 / (THETA ** j)                       # [32]
    ang = pos[:, None, :] * freq[None, :, None]     # [B, 32, S]
    cos_b = np.tile(np.cos(ang), (1, 4, 1)).astype(np.float32)  # [B, 128, S]
    sin_b = np.tile(np.sin(ang), (1, 4, 1)).astype(np.float32)
    # parity sign: +sin on parity-0 rows (r%64 < 32), -sin on parity-1
    sign = np.where((np.arange(128) % 64) < 32, 1.0, -1.0).astype(np.float32)
    sin_b = sin_b * sign[None, :, None]

    # causal triangle for the 128-wide diagonal band: tri[k, q] = NEG
    # where q < k (q measured from the tile's first in-band column)
    kk = np.arange(128)[:, None]
    qq = np.arange(128)[None, :]
    tri_np = np.where(qq < kk, NEG, 0.0).astype(ml_dtypes.bfloat16)
    eye_np = np.eye(128, dtype=ml_dtypes.bfloat16)
    # rden broadcast selectors; head h lives at (partition, block)
    # dslot[h], with ones over out-rows 64*(h%2)..64*(h%2)+64
    sel_np = np.zeros((128, 2, 128), dtype=ml_dtypes.bfloat16)
    for pr in range(2):
        for u in range(2):
            sel_np[32 * (2 * pr + u), pr, 64 * u:64 * u + 64] = 1.0
    sel_np = sel_np.reshape(128, 256)

    # head-major RoPE permutation within each core's 256 d_out rows:
    # e' = 128*(h//2) + 64*(h%2) + 32*p + j  <-  head h, component 2j+p
    perm = np.empty(E, dtype=np.int64)
    for h in range(HC):
        for p in range(2):
            for jj in range(32):
                perm[128 * (h // 2) + 64 * (h % 2) + 32 * p + jj] = (
                    64 * h + 2 * jj + p)

    bf = ml_dtypes.bfloat16

    def pack_w(wT, blocks):  # [D_in, F] -> [128, blocks, F] (p,c,f)
        d_in, f = wT.shape
        return np.ascontiguousarray(
            wT.reshape(blocks, 128, f).transpose(1, 0, 2)).astype(bf)

    in_maps = []
    for core in range(8):
        b, g = core // 4, core % 4
        rows = slice(E * g, E * (g + 1))
        wq_c = Wq[rows][perm] * (1.0 / np.sqrt(DK))
        wk_c = Wk[rows][perm]
        # x packed [128, chunk, dc, s']: p,c,dc,s -> x[b].T[dc*128+p,
        # c*SC+s] so each (p, c) DMA segment is 8*SC*2 = 8KB contiguous
        xb = x[b].T.reshape(8, 128, NSC, SC)
        x_pack = np.ascontiguousarray(xb.transpose(1, 2, 0, 3)).astype(bf)
        in_maps.append({
            "xT": x_pack,
            "wqT": pack_w(wq_c.T, 8),
            "wkT": pack_w(wk_c.T, 8),
            "wvT": pack_w(Wv[rows].T, 8),
            "woT": pack_w(Wo[:, rows].T, 2),
            "cosT": cos_b[b],
            "sinT": sin_b[b],
            "eye": eye_np,
            "tri": tri_np,
            "sel": sel_np,
        })
    return in_maps


def _run(in_maps, trace=False, trace_kwargs=None):
    global _COMPILED
    if _COMPILED is None:
        _COMPILED = _build()
    return run_bass_kernel_spmd(
        _COMPILED, in_maps, list(range(8)), trace=trace,
        **(trace_kwargs or {}))


def _gather(results):
    out = np.empty((B, S, D), dtype=np.float32)
    for b in range(B):
        acc = results[4 * b]["out"].astype(np.float32)
        for g in range(1, 4):
            acc = acc + results[4 * b + g]["out"].astype(np.float32)
        out[b] = acc
    return out


def kernel(x, Wq, Wk, Wv, Wo, token_positions):
    res = _run(_host_inputs(x, Wq, Wk, Wv, Wo, token_positions))
    return _gather(res.results)


def bench(x, Wq, Wk, Wv, Wo, token_positions):
    """Like kernel() but profiles on HW; returns (out, exec_time_ns)."""
    import types

    try:  # register the NTFF hook if the image's antenv lacks it
        from antenv import axon_hooks  # noqa: F401
    except ImportError:
        m = types.ModuleType("antenv.axon_hooks")
        from trn_agent_boot.trn_boot import _ntff_profile_via_ctypes
        hook = _ntff_profile_via_ctypes("/opt/axon/libaxon_pjrt.so")
        m.get_axon_ntff_profile_hook = lambda: hook
        m.set_axon_ntff_profile_hook = lambda h: None
        sys.modules["antenv.axon_hooks"] = m
        import antenv
        antenv.axon_hooks = m

    res = _run(_host_inputs(x, Wq, Wk, Wv, Wo, token_positions), trace=True)
    return _gather(res.results), res.exec_time_ns


# revision 37
# speedup vs baseline: 1.1029x; 1.1029x over previous
"""Multi-head causal attention with RoPE on 8 Trainium2 NeuronCores.

Problem: x[2, 2048, 1024], 16 heads, d_k=64, RoPE(theta=1e4), causal,
weights W{q,k,v,o}[1024, 1024] stored [d_out, d_in].

Sharding: 2 batches x 4 head-groups -> 8 cores. Core c handles batch
c//4, heads 4*(c%4) .. 4*(c%4)+4. Each core computes its 4 heads'
attention plus the partial o_proj for its head columns; the host sums
the 4 partials per batch (the "all-reduce after o_proj").

Two-phase structure (v3), one PSUM pool, banks rotating via tags:
  Phase 1 (all 4 chunks): Q/K projection psum pairs on SA/SB
    ([128,2,SC], 2 banks each), V on AV0/AV1. Chunk c+1's projection
    matmuls wait only on chunk c's RoPE psum muls (SA/SB WAR), so the
    PE pipelines across chunks. RoPE cross-parity swaps are SBUF
    copies split 3:1 ACT:DVE per tile (GpSimd measured ~1.9us/copy —
    unusable).
  Phase 2 (kt loops qc=0..3): score psum per head-pair on SA/SB with
    ONE exp activation per pair per k-tile ([128, 2, W] across both
    banks), A/B staggered so scores(kt+1) overlap exp(kt); attn@V
    accumulates into AV0..AV3, lagging one kt. The denominator
    broadcast (rbp) and the previous chunk's o_proj psum ride AV2/AV3
    after normalize frees them, so the SA/SB score ring is never
    blocked; o_proj(qc-1) is emitted after qc's first two kt
    emissions so ACT keeps streaming exps across the qc boundary.
    Output stores stream per 128-row s-tile as bf16.
"""

import sys

if "/opt/trn_rl_repo" not in sys.path:
    sys.path.insert(0, "/opt/trn_rl_repo")

import numpy as np

import concourse.bass as bass
import concourse.mybir as mybir
import concourse.tile as tile
from concourse import bacc
from concourse.bass_utils import run_bass_kernel_spmd

F32 = mybir.dt.float32
BF16 = mybir.dt.bfloat16
EXP = mybir.ActivationFunctionType.Exp

B = 2
S = 2048
D = 1024
H = 16
DK = 64
HC = 4          # heads per core
E = HC * DK     # 256 d_out columns per core
THETA = 10000.0
SC = 512        # seq chunk (psum free dim)
NSC = S // SC   # 4
NST = S // 128  # 16 s-tiles
NEG = -1.0e30

_COMPILED = None


def _build():
    nc = bacc.Bacc("TRN2", target_bir_lowering=False, debug=False, num_devices=8)

    # weights/x are pre-packed host-side into the exact SBUF layouts so
    # every DMA segment is multi-KB contiguous (the (c p) rearrange
    # loads had 512B segments and cost a ~20us lead-in)
    xT = nc.dram_tensor("xT", [128, NSC, 8, SC], BF16, kind="ExternalInput")
    wqT = nc.dram_tensor("wqT", [128, 8, E], BF16, kind="ExternalInput")
    wkT = nc.dram_tensor("wkT", [128, 8, E], BF16, kind="ExternalInput")
    wvT = nc.dram_tensor("wvT", [128, 8, E], BF16, kind="ExternalInput")
    woT = nc.dram_tensor("woT", [128, 2, D], BF16, kind="ExternalInput")
    cosT = nc.dram_tensor("cosT", [128, S], F32, kind="ExternalInput")
    sinT = nc.dram_tensor("sinT", [128, S], F32, kind="ExternalInput")
    eye = nc.dram_tensor("eye", [128, 128], BF16, kind="ExternalInput")
    tri = nc.dram_tensor("tri", [128, 128], BF16, kind="ExternalInput")
    sel = nc.dram_tensor("sel", [128, 256], BF16, kind="ExternalInput")
    out_d = nc.dram_tensor("out", [S, D], BF16, kind="ExternalOutput")

    with tile.TileContext(nc) as tc:
        with (
            tc.tile_pool(name="const", bufs=1) as const,
            tc.tile_pool(name="persist", bufs=1) as persist,
            tc.tile_pool(name="xp", bufs=2) as xp,
            tc.tile_pool(name="ropet", bufs=2) as ropet,
            tc.tile_pool(name="cspool", bufs=2) as cspool,
            tc.tile_pool(name="expool", bufs=3) as expool,
            tc.tile_pool(name="sopool", bufs=3) as sopool,
            tc.tile_pool(name="ps", bufs=1, space="PSUM") as ps,
        ):
            # ---- constant loads ------------------------------------
            wq_sb = const.tile([128, 8, E], BF16)
            wk_sb = const.tile([128, 8, E], BF16)
            wv_sb = const.tile([128, 8, E], BF16)
            wo_sb = const.tile([128, 2, D], BF16)
            eye_sb = const.tile([128, 128], BF16)
            tri_sb = const.tile([128, 128], BF16)
            sel_sb = const.tile([128, 2, 128], BF16)
            nc.sync.dma_start(wq_sb[:], wqT[:])

            # ---- persistent activations ----------------------------
            qA = persist.tile([128, S], BF16)   # heads 0,1 (rows 64h+32p+j)
            qB = persist.tile([128, S], BF16)   # heads 2,3
            kA = persist.tile([128, S], BF16)
            kB = persist.tile([128, S], BF16)
            v_sb = persist.tile([128, NST, HC * 65], BF16)
            ao = persist.tile([128, 2, S], BF16)    # o_proj lhsT
            # den slot for head h: (partition, block) = (32h, 0) for
            # h<3, (0, 1) for h=3 — matmul bases must be 0/32/64.
            den = persist.tile([128, SC], F32)
            rden = persist.tile([128, SC], F32)
            rdenb = persist.tile([128, SC], BF16)
            dume = persist.tile([128, 1], F32)

            v3 = v_sb[:].rearrange("p t (h c) -> p t h c", c=65)
            # ones column built on-device (a DMA of this strided
            # region degenerates to 8192 two-byte descriptors that
            # congest the queues for ~10us at startup)
            nc.vector.memset(v3[:, :, :, 64:65], 1.0)
            nc.vector.memset(den[:], 1.0)  # unused rows stay recip-safe
            # prefetch the exp table-set load during the proj phase
            nc.scalar.activation(dume[:], den[:, 0:1], EXP)

            qk = ((qA, kA), (qB, kB))
            x_last = []

            def proj_chunk(c):
                """QKV projections + RoPE + V layout for chunk c."""
                sl = slice(SC * c, SC * (c + 1))
                # chunk-0 triggers spread across engine queues so they
                # issue in parallel (Sync serializes at ~0.7-7us each)
                x_sb = xp.tile([128, 8, SC], BF16, name=f"x_{c}", tag="x")
                (nc.scalar if c == 0 else nc.sync).dma_start(
                    x_sb[:], xT[:, c])
                cs_sb = cspool.tile([128, SC], F32, name=f"cos_{c}",
                                    tag="cos")
                sn_sb = cspool.tile([128, SC], F32, name=f"sin_{c}",
                                    tag="sin")
                (nc.gpsimd if c == 0 else nc.sync).dma_start(
                    cs_sb[:], cosT[:, sl])
                (nc.gpsimd if c == 0 else nc.sync).dma_start(
                    sn_sb[:], sinT[:, sl])
                if c == 0:
                    nc.gpsimd.dma_start(wk_sb[:], wkT[:])
                    nc.gpsimd.dma_start(wv_sb[:], wvT[:])
                elif c == 1:
                    nc.sync.dma_start(eye_sb[:], eye[:])
                    nc.sync.dma_start(tri_sb[:], tri[:])
                    nc.sync.dma_start(
                        sel_sb[:],
                        sel[:].rearrange("p (c q) -> p c q", c=2))
                    nc.sync.dma_start(wo_sb[:], woT[:])

                pq = [ps.tile([128, SC], F32, name=f"pq{t}_{c}",
                              tag=f"SC{t}") for t in range(2)]
                pk = [ps.tile([128, SC], F32, name=f"pk{t}_{c}",
                              tag=f"SC{2 + t}") for t in range(2)]
                for psrc, w_sb in ((pq, wq_sb), (pk, wk_sb)):
                    for t in range(2):
                        es = slice(128 * t, 128 * (t + 1))
                        for dc in range(8):
                            nc.tensor.matmul(
                                psrc[t][:], w_sb[:, dc, es],
                                x_sb[:, dc, :],
                                start=(dc == 0), stop=(dc == 7))

                # RoPE: row = 64*(h%2) + 32*parity + j within each tile.
                # sinT carries +sin on parity-0 rows and -sin on
                # parity-1 rows, so after swapping 32-row blocks of
                # t1 = psum*sinAlt within each 64-row head block:
                #   t1s[p0] = -x2*s, t1s[p1] = +x1*s
                # and the combine is ONE aligned add: out = t0+t1s.
                # Cross-partition swaps are SBUF copies, 3 ACT + 1 DVE
                # per tile (GpSimd measured ~1.9us/copy — unusable).
                C = cs_sb[:]
                Sn = sn_sb[:]
                for name, psrc, dsts in (
                    ("q", pq, (qA, qB)),
                    ("k", pk, (kA, kB)),
                ):
                    for t in range(2):
                        dst = dsts[t]
                        t0 = ropet.tile([128, SC], F32,
                                        name=f"t0{name}{t}_{c}",
                                        tag=f"t0{name}{t}")
                        t1 = ropet.tile([128, SC], F32,
                                        name=f"t1{name}{t}_{c}",
                                        tag=f"t1{name}{t}")
                        t1s = ropet.tile([128, SC], F32,
                                         name=f"t1s{name}{t}_{c}",
                                         tag=f"t1s{name}{t}")
                        nc.vector.tensor_mul(t0[:], psrc[t][:], C)
                        nc.vector.tensor_mul(t1[:], psrc[t][:], Sn)
                        for bb in range(4):
                            dsl = slice(32 * bb, 32 * bb + 32)
                            ssl2 = slice(32 * (bb ^ 1), 32 * (bb ^ 1) + 32)
                            if bb == 3:
                                nc.vector.tensor_copy(
                                    t1s[dsl, :], t1[ssl2, :])
                            else:
                                nc.scalar.copy(t1s[dsl, :], t1[ssl2, :])
                        nc.vector.tensor_add(dst[:, sl], t0[:], t1s[:])

                if c < NSC - 1:
                    pv_chunk(c, x_sb, on_act=True)
                else:
                    x_last.append(x_sb)  # pv(3) deferred into phase 2

            def pv_chunk(c, x_sb, on_act):
                """V projection into [k, h*65+dk] layout (ones preset).

                The last chunk's pv is emitted inside attention_chunk(0)
                so its matmuls fill the phase-1 -> phase-2 PE gap (which
                otherwise trips the HAM clock gate for ~24us)."""
                pv = [ps.tile([128, 2, 256], F32, name=f"pv{t}_{c}",
                              tag=f"AV{t}") for t in range(2)]
                for st in range(4):
                    ssl = slice(128 * st, 128 * (st + 1))
                    for dc in range(8):
                        nc.tensor.matmul(
                            pv[st // 2][:, st % 2, :],
                            x_sb[:, dc, ssl], wv_sb[:, dc, :],
                            start=(dc == 0), stop=(dc == 7))
                for st in range(4):
                    cp = nc.scalar.copy if on_act else nc.vector.tensor_copy
                    cp(v3[:, 4 * c + st, :, 0:64],
                       pv[st // 2][:, st % 2, :]
                       .rearrange("p (h c) -> p h c", c=64))

            def o_proj_chunk(c, tags=("AV2", "AV3")):
                """o_proj partial for chunk c's 4 s-tiles (ao ready).

                po psum rides the given freed tags (AV2/AV3 inside the
                kt loops; all six free tags for the final chunk) so the
                score ring never waits on it and the po->evac chain is
                deep enough to pipeline."""
                for i in range(4):
                    stg = 4 * c + i
                    ssl = slice(128 * stg, 128 * (stg + 1))
                    for dc in range(2):
                        j = 2 * i + dc
                        po = ps.tile([128, SC], F32,
                                     name=f"po_{stg}_{dc}",
                                     tag=tags[j % len(tags)])
                        for pr in range(2):
                            nc.tensor.matmul(
                                po[:], ao[:, pr, ssl],
                                wo_sb[:, pr, 512 * dc:512 * (dc + 1)],
                                start=(pr == 0), stop=(pr == 1))
                        so = sopool.tile([128, SC], BF16,
                                         name=f"so_{stg}_{dc}", tag="so")
                        if len(tags) > 2 and j % 2 == 1:
                            nc.scalar.copy(so[:], po[:])  # tail only
                        else:
                            nc.vector.tensor_copy(so[:], po[:])
                        nc.sync.dma_start(
                            out_d[ssl, 512 * dc:512 * (dc + 1)], so[:])

            def attention_chunk(qc):
                """Scores + exp + attn@V + normalize for q-chunk qc.

                o_proj for qc-1 is emitted after this chunk's first
                two kt emissions, so ACT keeps streaming exps across
                the qc boundary while the PE runs the po matmuls."""
                qsl = slice(SC * qc, SC * (qc + 1))
                nkt = 4 * qc + 4
                # avs allocated lazily AFTER o_proj(qc-1)'s po tiles so
                # the AV2/3 tag ring is: avs(qc-1), rbp(qc-1),
                # po(qc-1), avs(qc) — po never waits on this kt loop.
                avs = []

                def get_avs():
                    if not avs:
                        avs.extend(
                            ps.tile([128, SC], F32, name=f"av{h}_{qc}",
                                    tag=f"AV{h}") for h in range(HC))
                    return avs

                # diag k-tiles first: ends the chunk on uniform
                # full-width exps (measured ~3us better than natural
                # order — the shrinking diag tail skews the per-head
                # pipelines at the chunk boundary)
                kt_order = list(range(4 * qc, 4 * qc + 4)) + list(range(4 * qc))
                first_kt, last_kt = kt_order[0], kt_order[-1]
                prev = None

                def attn_v(pkt, pw, pexs, final=False):
                    for h in range(HC):
                        nc.tensor.matmul(
                            get_avs()[h][0:65, pw:SC],
                            v_sb[:, pkt, 65 * h:65 * h + 65],
                            pexs[h][:, pw:SC],
                            start=(pkt == first_kt), stop=(pkt == last_kt))
                        if final:
                            # per-head normalize pieces right behind
                            # the head's last attn@V. ACT may only be
                            # used on the very last chunk (ACT is the
                            # phase-2 bottleneck; after its last exp
                            # it is free and shortens the tail chain)
                            act_ok = qc == NSC - 1
                            u, pr = h % 2, h // 2
                            if act_ok and h % 2 == 0:
                                nc.scalar.copy(
                                    den[32 * h:32 * h + 1, :],
                                    get_avs()[h][64:65, :])
                                nc.vector.tensor_copy(
                                    ao[64 * u:64 * u + 64, pr, qsl],
                                    get_avs()[h][0:64, :])
                            elif act_ok:
                                nc.vector.tensor_copy(
                                    den[32 * h:32 * h + 1, :],
                                    get_avs()[h][64:65, :])
                                nc.scalar.copy(
                                    ao[64 * u:64 * u + 64, pr, qsl],
                                    get_avs()[h][0:64, :])
                            else:
                                nc.vector.tensor_copy(
                                    den[32 * h:32 * h + 1, :],
                                    get_avs()[h][64:65, :])
                                nc.vector.tensor_copy(
                                    ao[64 * u:64 * u + 64, pr, qsl],
                                    get_avs()[h][0:64, :])

                for ki, kt in enumerate(kt_order):
                    ksl = slice(128 * kt, 128 * (kt + 1))
                    diag = kt >= 4 * qc
                    w = 128 * (kt - 4 * qc) if diag else 0
                    exs = []
                    # one 1-bank score tile + one exp per head: four
                    # independent exp->score chains keep ACT saturated
                    # no matter how the scheduler orders them (pair
                    # batching had a 2-bank WAR ping-pong the scheduler
                    # kept serializing)
                    for h in range(HC):
                        q_t, k_t = qk[h // 2]
                        rsl = slice(64 * (h % 2), 64 * (h % 2) + 64)
                        sc = ps.tile([128, SC], F32,
                                     name=f"sc{h}_{qc}_{kt}",
                                     tag=f"SC{h}")
                        nc.tensor.matmul(
                            sc[:, w:SC], k_t[rsl, ksl],
                            q_t[rsl, qsl][:, w:SC],
                            start=True, stop=not diag)
                        if diag:
                            nc.tensor.matmul(
                                sc[:, w:w + 128], eye_sb[:],
                                tri_sb[:], start=False, stop=True)
                        ex = expool.tile([128, SC], BF16,
                                         name=f"ex{h}_{qc}_{kt}",
                                         tag=f"ex{h}")
                        nc.scalar.activation(
                            ex[:, w:SC], sc[:, w:SC], EXP)
                        exs.append(ex)
                        if h == HC - 1 and prev is not None:
                            attn_v(*prev)
                    prev = (kt, w, exs)
                    if ki == 0:
                        if qc == 0:
                            pv_chunk(NSC - 1, x_last[0], on_act=False)
                        else:
                            o_proj_chunk(qc - 1)
                attn_v(*prev, final=True)

                nc.vector.reciprocal_approx_fast(rden[:], den[:])
                nc.vector.tensor_copy(rdenb[:], rden[:])
                for pr in range(2):
                    rbp = ps.tile([128, SC], F32, name=f"rbp_{qc}_{pr}",
                                  tag=f"AV{2 + pr}")
                    nc.tensor.matmul(
                        rbp[:], sel_sb[:, pr, :], rdenb[:],
                        start=True, stop=True)
                    nc.vector.tensor_mul(
                        ao[:, pr, qsl], ao[:, pr, qsl], rbp[:])

            for c in range(NSC):
                proj_chunk(c)
            for qc in range(NSC):
                attention_chunk(qc)
            o_proj_chunk(
                NSC - 1,
                tags=("AV2", "AV3", "SC0", "SC1", "SC2", "SC3"))

    nc.compile()
    return nc


def _host_inputs(x, Wq, Wk, Wv, Wo, token_positions):
    """Build the 8 per-core input maps (all host-side numpy prep)."""
    import ml_dtypes

    x = np.asarray(x, dtype=np.float32)
    Wq = np.asarray(Wq, dtype=np.float32)
    Wk = np.asarray(Wk, dtype=np.float32)
    Wv = np.asarray(Wv, dtype=np.float32)
    Wo = np.asarray(Wo, dtype=np.float32)
    pos = np.asarray(token_positions, dtype=np.int64)

    # RoPE tables per batch: row 32a+j -> cos/sin(pos[s] * freq[j])
    j = np.arange(0, DK, 2, dtype=np.float64) / DK
    freq = 1.0 / (THETA ** j)                       # [32]
    ang = pos[:, None, :] * freq[None, :, None]     # [B, 32, S]
    cos_b = np.tile(np.cos(ang), (1, 4, 1)).astype(np.float32)  # [B, 128, S]
    sin_b = np.tile(np.sin(ang), (1, 4, 1)).astype(np.float32)
    # parity sign: +sin on parity-0 rows (r%64 < 32), -sin on parity-1
    sign = np.where((np.arange(128) % 64) < 32, 1.0, -1.0).astype(np.float32)
    sin_b = sin_b * sign[None, :, None]

    # causal triangle for the 128-wide diagonal band: tri[k, q] = NEG
    # where q < k (q measured from the tile's first in-band column)
    kk = np.arange(128)[:, None]
    qq = np.arange(128)[None, :]
    tri_np = np.where(qq < kk, NEG, 0.0).astype(ml_dtypes.bfloat16)
    eye_np = np.eye(128, dtype=ml_dtypes.bfloat16)
    # rden broadcast selectors; head h lives at (partition, block)
    # dslot[h], with ones over out-rows 64*(h%2)..64*(h%2)+64
    sel_np = np.zeros((128, 2, 128), dtype=ml_dtypes.bfloat16)
    for pr in range(2):
        for u in range(2):
            sel_np[32 * (2 * pr + u), pr, 64 * u:64 * u + 64] = 1.0
    sel_np = sel_np.reshape(128, 256)

    # head-major RoPE permutation within each core's 256 d_out rows:
    # e' = 128*(h//2) + 64*(h%2) + 32*p + j  <-  head h, component 2j+p
    perm = np.empty(E, dtype=np.int64)
    for h in range(HC):
        for p in range(2):
            for jj in range(32):
                perm[128 * (h // 2) + 64 * (h % 2) + 32 * p + jj] = (
                    64 * h + 2 * jj + p)

    bf = ml_dtypes.bfloat16

    def pack_w(wT, blocks):  # [D_in, F] -> [128, blocks, F] (p,c,f)
        d_in, f = wT.shape
        return np.ascontiguousarray(
            wT.reshape(blocks, 128, f).transpose(1, 0, 2)).astype(bf)

    in_maps = []
    for core in range(8):
        b, g = core // 4, core % 4
        rows = slice(E * g, E * (g + 1))
        wq_c = Wq[rows][perm] * (1.0 / np.sqrt(DK))
        wk_c = Wk[rows][perm]
        # x packed [128, chunk, dc, s']: p,c,dc,s -> x[b].T[dc*128+p,
        # c*SC+s] so each (p, c) DMA segment is 8*SC*2 = 8KB contiguous
        xb = x[b].T.reshape(8, 128, NSC, SC)
        x_pack = np.ascontiguousarray(xb.transpose(1, 2, 0, 3)).astype(bf)
        in_maps.append({
            "xT": x_pack,
            "wqT": pack_w(wq_c.T, 8),
            "wkT": pack_w(wk_c.T, 8),
            "wvT": pack_w(Wv[rows].T, 8),
            "woT": pack_w(Wo[:, rows].T, 2),
            "cosT": cos_b[b],
            "sinT": sin_b[b],
            "eye": eye_np,
            "tri": tri_np,
            "sel": sel_np,
        })
    return in_maps


def _run(in_maps, trace=False, trace_kwargs=None):
    global _COMPILED
    if _COMPILED is None:
        _COMPILED = _build()
    return run_bass_kernel_spmd(
        _COMPILED, in_maps, list(range(8)), trace=trace,
        **(trace_kwargs or {}))


def _gather(results):
    out = np.empty((B, S, D), dtype=np.float32)
    for b in range(B):
        acc = results[4 * b]["out"].astype(np.float32)
        for g in range(1, 4):
            acc = acc + results[4 * b + g]["out"].astype(np.float32)
        out[b] = acc
    return out


def kernel(x, Wq, Wk, Wv, Wo, token_positions):
    res = _run(_host_inputs(x, Wq, Wk, Wv, Wo, token_positions))
    return _gather(res.results)


def bench(x, Wq, Wk, Wv, Wo, token_positions):
    """Like kernel() but profiles on HW; returns (out, exec_time_ns)."""
    import types

    try:  # register the NTFF hook if the image's antenv lacks it
        from antenv import axon_hooks  # noqa: F401
    except ImportError:
        m = types.ModuleType("antenv.axon_hooks")
        from trn_agent_boot.trn_boot import _ntff_profile_via_ctypes
        hook = _ntff_profile_via_ctypes("/opt/axon/libaxon_pjrt.so")
        m.get_axon_ntff_profile_hook = lambda: hook
        m.set_axon_ntff_profile_hook = lambda h: None
        sys.modules["antenv.axon_hooks"] = m
        import antenv
        antenv.axon_hooks = m

    res = _run(_host_inputs(x, Wq, Wk, Wv, Wo, token_positions), trace=True)
    return _gather(res.results), res.exec_time_ns
